# revision 1
# baseline (speedup 1.0000x reference)
"""Trainium2 Bass kernel: per-node mean over gathered hyperedge embeddings.

out[n, :] = mean_k table[idx[n, k], :]   (idx: [100000, 32], table: [500000, 128])

Strategy: nodes sharded across 8 cores; fp16 table (scale 1/32 folded in,
exact) replicated per core. The heavy lifting uses InstDMAGatherAnt (fast
SWDGE descriptor generation, ~0.34ns/row vs ~1us/instruction for generic
indirect DMA), whose int16 indices force a two-pass scheme:

  pass 1: per chunk of 896 nodes (28672 lookups), bucket the lookups by
          table-row >> 15 (16 buckets of 32768 rows; indices bucket-local
          fit int16) and gather each bucket's rows HBM->SBUF staging with
          one dma_gather per bucket (in_ap = bucket slice of the table).
  pass 2: SBUF-source transpose dma_gather re-orders staging tokens into
          node-major columns ([128=embed, tokens]); positions within the
          staging chunk also fit int16.
  reduce: DVE tensor_reduce sums each node's 32 consecutive columns;
          output stored transposed [128, nodes], un-transposed on host.

Per-call index counts are padded to a per-call constant (max across cores,
rounded to 128; pad entries gather row 0 of the bucket and are never
referenced by pass 2), so every num_idxs/num_idxs_reg is compile-time.
Staging tiles are explicitly ping-ponged (pool-rotated tiles with sliced
gather writes trip a tile-scheduler deadlock).
"""

import numpy as np

import concourse.bass as bass
import concourse.tile as tile
from concourse import bacc, mybir
from concourse.bass_utils import run_bass_kernel_spmd

P = 128
N_EDGES = 500000
EMBED = 128
N_NODES = 100000
DEGREE = 32
N_CORES = 8
NODES_PER_CORE = N_NODES // N_CORES  # 12500
PAD_NODES = 12544  # 14 chunks x 896 nodes
CHUNKS = 14
NPC = 896  # nodes per chunk
CPOS = NPC * DEGREE  # 28672 positions per chunk
NBUCK = 16
BUCK = 32768  # rows per bucket (last bucket: 8480)
SUB = 4  # pass-2 sub-calls per chunk
N2S = CPOS // SUB  # 7168
MAX_SLOTS = 254  # staging slots cap (32512 positions, int16-safe)
NQ = 2  # SWDGE queues
PIECE = 1024  # pass-1 max idx per call (64 desc/engine, at the packet cap)
PIECE2 = 896  # pass-2 (transpose) max idx per call

_cache = {}


def _preprocess(idx):
    """idx [100000, 32] int32 -> per-core index streams + shared call sizes.

    Returns (C, i1_maps, i2_maps):
      C[ch][b]   - pass-1 call size (multiple of 128, same for all cores)
      i1_maps[c] - [128, TOT1/16] int16 pass-1 streams (bucket-local rows)
      i2_maps[c] - [128, CHUNKS*CPOS/16] int16 pass-2 staging positions
    """
    cores = []
    for c in range(N_CORES):
        shard = idx[c * NODES_PER_CORE : (c + 1) * NODES_PER_CORE]
        pad = np.zeros((PAD_NODES, DEGREE), np.int32)
        pad[:NODES_PER_CORE] = shard
        cores.append(pad.reshape(CHUNKS, CPOS))

    counts = np.zeros((N_CORES, CHUNKS, NBUCK), np.int64)
    orders = {}
    for c in range(N_CORES):
        for ch in range(CHUNKS):
            v = cores[c][ch]
            b = v >> 15
            orders[(c, ch)] = (np.argsort(b, kind="stable"), b)
            counts[c, ch] = np.bincount(b, minlength=NBUCK)

    # shared per-call constants
    C = np.zeros((CHUNKS, NBUCK), np.int64)
    for ch in range(CHUNKS):
        m = counts[:, ch, :].max(axis=0)
        C[ch] = ((m + 127) // 128) * 128
        assert C[ch].sum() <= MAX_SLOTS * P, (ch, C[ch].sum())

    def wrap16(a):
        n = a.shape[0]
        return np.tile(a.reshape(n // 16, 16).T, (8, 1))

    i1_maps, i2_maps = [], []
    for c in range(N_CORES):
        blocks1, blocks2 = [], []
        for ch in range(CHUNKS):
            order, b = orders[(c, ch)]
            v = cores[c][ch]
            cnt = counts[c, ch]
            starts = np.concatenate([[0], np.cumsum(cnt)[:-1]])
            S128 = np.concatenate([[0], np.cumsum(C[ch])[:-1]])  # position base
            # pass-1 streams (bucket-local indices, padded with 0)
            vs = v[order]
            bs = b[order]
            for bk in range(NBUCK):
                s, e = starts[bk], starts[bk] + cnt[bk]
                lidx = (vs[s:e] - (bk << 15)).astype(np.int16)
                padded = np.zeros(C[ch][bk], np.int16)
                padded[: cnt[bk]] = lidx
                blocks1.append(padded)
            # pass-2 positions: for sorted pair r: pos = S128[b] + rank-in-bucket
            rank = np.arange(CPOS) - starts[bs]
            pos_sorted = (S128[bs] + rank).astype(np.int16)
            pos = np.empty(CPOS, np.int16)
            pos[order] = pos_sorted
            blocks2.append(pos)
        i1_maps.append(np.ascontiguousarray(wrap16(np.concatenate(blocks1))))
        i2_maps.append(np.ascontiguousarray(wrap16(np.concatenate(blocks2))))
    return C, i1_maps, i2_maps


def _build(C):
    key = C.tobytes()
    if _cache.get("key") == key:
        return _cache["nc"]
    tot1_16 = int(C.sum()) // 16
    ch16 = [int(C[ch].sum()) // 16 for ch in range(CHUNKS)]
    max16 = max(ch16)

    nc = bacc.Bacc(
        "TRN2",
        target_bir_lowering=False,
        debug=False,
        enable_asserts=False,
        num_devices=N_CORES,
        num_swdge_queues=NQ,
    )
    t16 = nc.dram_tensor(
        "t16", [N_EDGES, EMBED], mybir.dt.float16, kind="ExternalInput"
    ).ap()
    i1 = nc.dram_tensor("i1", [P, tot1_16], mybir.dt.int16, kind="ExternalInput").ap()
    i2 = nc.dram_tensor(
        "i2", [P, CHUNKS * CPOS // 16], mybir.dt.int16, kind="ExternalInput"
    ).ap()
    out = nc.dram_tensor(
        "out", [P, PAD_NODES], mybir.dt.float32, kind="ExternalOutput"
    ).ap()

    with tile.TileContext(nc) as tc:
        with (
            tc.tile_pool(name="aux", bufs=1) as aux,
            tc.tile_pool(name="op", bufs=4) as op,
        ):
            stg = [
                aux.tile([P, MAX_SLOTS, EMBED], mybir.dt.float16, name=f"stg{i}")
                for i in range(2)
            ]
            g2 = [
                aux.tile([P, N2S], mybir.dt.float16, name=f"g2_{i}") for i in range(2)
            ]
            ia = [
                aux.tile([P, max16], mybir.dt.int16, name=f"ia{i}") for i in range(2)
            ]
            ib = [
                aux.tile([P, CPOS // 16], mybir.dt.int16, name=f"ib{i}")
                for i in range(2)
            ]
            off1 = 0
            qctr = [0]
            for ch in range(CHUNKS):
                s = stg[ch % 2]
                xa = ia[ch % 2]
                xb = ib[ch % 2]
                nc.sync.dma_start(
                    out=xa[:, : ch16[ch]], in_=i1[:, off1 : off1 + ch16[ch]]
                )
                nc.sync.dma_start(
                    out=xb[:], in_=i2[:, ch * (CPOS // 16) : (ch + 1) * (CPOS // 16)]
                )
                off1 += ch16[ch]
                boff = 0
                for bk in range(NBUCK):
                    cb = int(C[ch][bk])
                    rows = min(BUCK, N_EDGES - bk * BUCK)
                    poff = 0
                    while poff < cb:
                        piece = min(PIECE, cb - poff)
                        a = boff + poff
                        nc.gpsimd.dma_gather(
                            out_ap=s[:, a // 128 : (a + piece) // 128, :],
                            in_ap=t16[bk * BUCK : bk * BUCK + rows, :],
                            idxs_ap=xa[:, a // 16 : (a + piece) // 16],
                            num_idxs=piece,
                            num_idxs_reg=piece,
                            elem_size=EMBED,
                            queue_num=qctr[0] % NQ,
                        )
                        qctr[0] += 1
                        poff += piece
                    boff += cb
                for sb in range(SUB):
                    g = g2[(ch * SUB + sb) % 2]
                    for p2 in range(N2S // PIECE2):
                        nc.gpsimd.dma_gather(
                            out_ap=g[:, p2 * PIECE2 : (p2 + 1) * PIECE2].rearrange(
                                "p (c n) -> p c n", c=1
                            ),
                            in_ap=s[:].rearrange("p s d -> p (s d)"),
                            idxs_ap=xb[
                                :,
                                (sb * N2S + p2 * PIECE2) // 16 : (sb * N2S + (p2 + 1) * PIECE2) // 16,
                            ],
                            num_idxs=PIECE2,
                            num_idxs_reg=PIECE2,
                            elem_size=EMBED,
                            transpose=True,
                            sbuf_tokens_per_rank=128,
                            sbuf_free_dim_per_rank=2 * EMBED,
                            queue_num=qctr[0] % NQ,
                        )
                        qctr[0] += 1
                    o = op.tile([P, N2S // DEGREE], mybir.dt.float32)
                    nc.vector.tensor_reduce(
                        out=o[:],
                        in_=g[:].rearrange("p (n k) -> p n k", k=DEGREE),
                        axis=mybir.AxisListType.X,
                        op=mybir.AluOpType.add,
                    )
                    col = (ch * SUB + sb) * (N2S // DEGREE)
                    nc.sync.dma_start(
                        out=out[:, col : col + N2S // DEGREE], in_=o[:]
                    )
    nc.compile()
    _cache["key"] = key
    _cache["nc"] = nc
    return nc


def run(embedding_table, node_hyperedges, **spmd_kwargs):
    """Run on 8 cores; returns (full_output, BassKernelResults)."""
    table = np.asarray(embedding_table, dtype=np.float32) * np.float32(1.0 / 32.0)
    t16 = np.ascontiguousarray(table.astype(np.float16))
    idx = np.ascontiguousarray(np.asarray(node_hyperedges).astype(np.int32))
    assert t16.shape == (N_EDGES, EMBED)
    assert idx.shape == (N_NODES, DEGREE)

    C, i1_maps, i2_maps = _preprocess(idx)
    nc = _build(C)
    in_maps = [
        {"t16": t16, "i1": i1_maps[c], "i2": i2_maps[c]} for c in range(N_CORES)
    ]
    res = run_bass_kernel_spmd(nc, in_maps, list(range(N_CORES)), **spmd_kwargs)
    out = np.concatenate(
        [res.results[c]["out"].T[:NODES_PER_CORE] for c in range(N_CORES)], axis=0
    ).astype(np.float32)
    return out, res


def kernel(embedding_table, node_hyperedges):
    out, _ = run(embedding_table, node_hyperedges)
    return out



# revision 2
# speedup vs baseline: 1.8332x; 1.8332x over previous
"""Trainium2 Bass kernel: per-node mean over gathered hyperedge embeddings.

out[n, :] = mean_k table[idx[n, k], :]   (idx: [100000, 32], table: [500000, 128])

Strategy: nodes sharded across 8 cores; fp16 table (scale 1/32 folded in,
exact) replicated per core. The heavy lifting uses InstDMAGatherAnt (fast
SWDGE descriptor generation, ~0.34ns/row vs ~1us/instruction for generic
indirect DMA), whose int16 indices force a two-pass scheme:

  pass 1: per chunk of 896 nodes (28672 lookups), bucket the lookups by
          table-row >> 15 (16 buckets of 32768 rows; indices bucket-local
          fit int16) and gather each bucket's rows HBM->SBUF staging with
          one dma_gather per bucket (in_ap = bucket slice of the table).
  pass 2: SBUF-source transpose dma_gather re-orders staging tokens into
          node-major columns ([128=embed, tokens]); positions within the
          staging chunk also fit int16.
  reduce: DVE tensor_reduce sums each node's 32 consecutive columns;
          output stored transposed [128, nodes], un-transposed on host.

Per-call index counts are padded to a per-call constant (max across cores,
rounded to 128; pad entries gather row 0 of the bucket and are never
referenced by pass 2), so every num_idxs/num_idxs_reg is compile-time.
Staging tiles are explicitly ping-ponged (pool-rotated tiles with sliced
gather writes trip a tile-scheduler deadlock).
"""

import numpy as np

import concourse.bass as bass
import concourse.tile as tile
from concourse import bacc, mybir
from concourse.bass_utils import run_bass_kernel_spmd

P = 128
N_EDGES = 500000
EMBED = 128
N_NODES = 100000
DEGREE = 32
N_CORES = 8
NODES_PER_CORE = N_NODES // N_CORES  # 12500
PAD_NODES = 12544  # 14 chunks x 896 nodes
CHUNKS = 14
NPC = 896  # nodes per chunk
CPOS = NPC * DEGREE  # 28672 positions per chunk
NBUCK = 16
BUCK = 32768  # rows per bucket (last bucket: 8480)
SUB = 4  # pass-2 sub-calls per chunk
N2S = CPOS // SUB  # 7168
MAX_SLOTS = 254  # staging slots cap (32512 positions, int16-safe)
NQ = 4  # SWDGE queues (ucode MAX_SWDGE_QUEUES=4; queue q runs on Q7 cpu pair {2q,2q+1})
PIECE = 1024  # pass-1 max idx per call (64 desc/engine, at the packet cap)
PIECE2 = 896  # pass-2 (transpose) max idx per call

_cache = {}


def _preprocess(idx):
    """idx [100000, 32] int32 -> per-core index streams + shared call sizes.

    Returns (C, i1_maps, i2_maps):
      C[ch][b]   - pass-1 call size (multiple of 128, same for all cores)
      i1_maps[c] - [128, TOT1/16] int16 pass-1 streams (bucket-local rows)
      i2_maps[c] - [128, CHUNKS*CPOS/16] int16 pass-2 staging positions
    """
    cores = []
    for c in range(N_CORES):
        shard = idx[c * NODES_PER_CORE : (c + 1) * NODES_PER_CORE]
        pad = np.zeros((PAD_NODES, DEGREE), np.int32)
        pad[:NODES_PER_CORE] = shard
        cores.append(pad.reshape(CHUNKS, CPOS))

    counts = np.zeros((N_CORES, CHUNKS, NBUCK), np.int64)
    orders = {}
    for c in range(N_CORES):
        for ch in range(CHUNKS):
            v = cores[c][ch]
            b = v >> 15
            orders[(c, ch)] = (np.argsort(b, kind="stable"), b)
            counts[c, ch] = np.bincount(b, minlength=NBUCK)

    # shared per-call constants
    C = np.zeros((CHUNKS, NBUCK), np.int64)
    for ch in range(CHUNKS):
        m = counts[:, ch, :].max(axis=0)
        C[ch] = ((m + 127) // 128) * 128
        assert C[ch].sum() <= MAX_SLOTS * P, (ch, C[ch].sum())

    def wrap16(a):
        n = a.shape[0]
        return np.tile(a.reshape(n // 16, 16).T, (8, 1))

    i1_maps, i2_maps = [], []
    for c in range(N_CORES):
        blocks1, blocks2 = [], []
        for ch in range(CHUNKS):
            order, b = orders[(c, ch)]
            v = cores[c][ch]
            cnt = counts[c, ch]
            starts = np.concatenate([[0], np.cumsum(cnt)[:-1]])
            S128 = np.concatenate([[0], np.cumsum(C[ch])[:-1]])  # position base
            # pass-1 streams (bucket-local indices, padded with 0)
            vs = v[order]
            bs = b[order]
            for bk in range(NBUCK):
                s, e = starts[bk], starts[bk] + cnt[bk]
                lidx = (vs[s:e] - (bk << 15)).astype(np.int16)
                padded = np.zeros(C[ch][bk], np.int16)
                padded[: cnt[bk]] = lidx
                blocks1.append(padded)
            # pass-2 positions: for sorted pair r: pos = S128[b] + rank-in-bucket
            rank = np.arange(CPOS) - starts[bs]
            pos_sorted = (S128[bs] + rank).astype(np.int16)
            pos = np.empty(CPOS, np.int16)
            pos[order] = pos_sorted
            blocks2.append(pos)
        i1_maps.append(np.ascontiguousarray(wrap16(np.concatenate(blocks1))))
        i2_maps.append(np.ascontiguousarray(wrap16(np.concatenate(blocks2))))
    return C, i1_maps, i2_maps


def _build(C):
    key = C.tobytes()
    if _cache.get("key") == key:
        return _cache["nc"]
    tot1_16 = int(C.sum()) // 16
    ch16 = [int(C[ch].sum()) // 16 for ch in range(CHUNKS)]
    max16 = max(ch16)

    nc = bacc.Bacc(
        "TRN2",
        target_bir_lowering=False,
        debug=False,
        enable_asserts=False,
        num_devices=N_CORES,
        num_swdge_queues=NQ,
    )
    t16 = nc.dram_tensor(
        "t16", [N_EDGES, EMBED], mybir.dt.float16, kind="ExternalInput"
    ).ap()
    i1 = nc.dram_tensor("i1", [P, tot1_16], mybir.dt.int16, kind="ExternalInput").ap()
    i2 = nc.dram_tensor(
        "i2", [P, CHUNKS * CPOS // 16], mybir.dt.int16, kind="ExternalInput"
    ).ap()
    out = nc.dram_tensor(
        "out", [P, PAD_NODES], mybir.dt.float32, kind="ExternalOutput"
    ).ap()

    with tile.TileContext(nc) as tc:
        with (
            tc.tile_pool(name="aux", bufs=1) as aux,
            tc.tile_pool(name="op", bufs=4) as op,
        ):
            stg = [
                aux.tile([P, MAX_SLOTS, EMBED], mybir.dt.float16, name=f"stg{i}")
                for i in range(2)
            ]
            g2 = [
                aux.tile([P, N2S], mybir.dt.float16, name=f"g2_{i}") for i in range(2)
            ]
            ia = [
                aux.tile([P, max16], mybir.dt.int16, name=f"ia{i}") for i in range(2)
            ]
            ib = [
                aux.tile([P, CPOS // 16], mybir.dt.int16, name=f"ib{i}")
                for i in range(2)
            ]
            off1 = 0
            qctr = [0]
            for ch in range(CHUNKS):
                s = stg[ch % 2]
                xa = ia[ch % 2]
                xb = ib[ch % 2]
                nc.sync.dma_start(
                    out=xa[:, : ch16[ch]], in_=i1[:, off1 : off1 + ch16[ch]]
                )
                nc.sync.dma_start(
                    out=xb[:], in_=i2[:, ch * (CPOS // 16) : (ch + 1) * (CPOS // 16)]
                )
                off1 += ch16[ch]
                boff = 0
                for bk in range(NBUCK):
                    cb = int(C[ch][bk])
                    rows = min(BUCK, N_EDGES - bk * BUCK)
                    poff = 0
                    while poff < cb:
                        piece = min(PIECE, cb - poff)
                        a = boff + poff
                        nc.gpsimd.dma_gather(
                            out_ap=s[:, a // 128 : (a + piece) // 128, :],
                            in_ap=t16[bk * BUCK : bk * BUCK + rows, :],
                            idxs_ap=xa[:, a // 16 : (a + piece) // 16],
                            num_idxs=piece,
                            num_idxs_reg=piece,
                            elem_size=EMBED,
                            queue_num=qctr[0] % NQ,
                        )
                        qctr[0] += 1
                        poff += piece
                    boff += cb
                for sb in range(SUB):
                    g = g2[(ch * SUB + sb) % 2]
                    for p2 in range(N2S // PIECE2):
                        nc.gpsimd.dma_gather(
                            out_ap=g[:, p2 * PIECE2 : (p2 + 1) * PIECE2].rearrange(
                                "p (c n) -> p c n", c=1
                            ),
                            in_ap=s[:].rearrange("p s d -> p (s d)"),
                            idxs_ap=xb[
                                :,
                                (sb * N2S + p2 * PIECE2) // 16 : (sb * N2S + (p2 + 1) * PIECE2) // 16,
                            ],
                            num_idxs=PIECE2,
                            num_idxs_reg=PIECE2,
                            elem_size=EMBED,
                            transpose=True,
                            sbuf_tokens_per_rank=128,
                            sbuf_free_dim_per_rank=2 * EMBED,
                            queue_num=qctr[0] % NQ,
                        )
                        qctr[0] += 1
                    o = op.tile([P, N2S // DEGREE], mybir.dt.float32)
                    nc.vector.tensor_reduce(
                        out=o[:],
                        in_=g[:].rearrange("p (n k) -> p n k", k=DEGREE),
                        axis=mybir.AxisListType.X,
                        op=mybir.AluOpType.add,
                    )
                    col = (ch * SUB + sb) * (N2S // DEGREE)
                    nc.sync.dma_start(
                        out=out[:, col : col + N2S // DEGREE], in_=o[:]
                    )
    nc.compile()
    _cache["key"] = key
    _cache["nc"] = nc
    return nc


def run(embedding_table, node_hyperedges, **spmd_kwargs):
    """Run on 8 cores; returns (full_output, BassKernelResults)."""
    table = np.asarray(embedding_table, dtype=np.float32) * np.float32(1.0 / 32.0)
    t16 = np.ascontiguousarray(table.astype(np.float16))
    idx = np.ascontiguousarray(np.asarray(node_hyperedges).astype(np.int32))
    assert t16.shape == (N_EDGES, EMBED)
    assert idx.shape == (N_NODES, DEGREE)

    C, i1_maps, i2_maps = _preprocess(idx)
    nc = _build(C)
    in_maps = [
        {"t16": t16, "i1": i1_maps[c], "i2": i2_maps[c]} for c in range(N_CORES)
    ]
    res = run_bass_kernel_spmd(nc, in_maps, list(range(N_CORES)), **spmd_kwargs)
    out = np.concatenate(
        [res.results[c]["out"].T[:NODES_PER_CORE] for c in range(N_CORES)], axis=0
    ).astype(np.float32)
    return out, res


def kernel(embedding_table, node_hyperedges):
    out, _ = run(embedding_table, node_hyperedges)
    return out

